# revision 6
# baseline (speedup 1.0000x reference)
"""Multi-head attention (B=4, S=2048, D=1024, H=16) on 8 TRN2 NeuronCores.

Sharding: core c handles batch b=c//2 and head-group g=c%2 (8 heads, 512 of
the 1024 model dims).  Wq/Wk/Wv column-parallel, Wo row-parallel; the two
head-group partial outputs per batch are summed on the host (no collectives).

Per-core dataflow (all matmuls in f32r = TF32, fp32 PSUM accumulate):
  phase 1: Q.T = (Wq/8) @ x.T   [512,2048]   (bias via K=1 ones-row matmul)
           K.T = Wk @ x.T       [512,2048]
           V   = x @ Wv.T       [2048,512]   stored head-interleaved with a
                                             ones column: [128,16,8*65]
  phase 2 (per 512-wide q-block, per head):
           scoresT[k,q] = K_h.T.T @ Q_h.T    (K=64 contraction)
           expT = exp(scoresT) on ACT (no max subtraction needed: |s|<~10)
           mixed 128-chunks multiplied by 0/1 mask tiles; fully-masked
           column windows simply not computed (causal skipping)
           raw[65,512] += [V_h|1].T @ expT   (row 64 = softmax denominator)
           attnT[h] = raw[0:64] * (1/raw[64]) broadcast (gpsimd bcast + DVE)
  phase 3 (per q-block): outT += Wo_g.T.T @ attnT_cat -> [1024,2048] partial
Host: out[b] = (partial_g0 + partial_g1).T + bo
"""

import numpy as np
from contextlib import ExitStack

B = 4
S = 2048
D = 1024
H = 16
DK = 64
G = 2                 # head groups
HL = H // G           # heads per core = 8
DL = D // G           # local head dims = 512
QB = 512              # q-block width
CH = 128              # chunk / k-tile width
NKT = S // CH         # 16 k-tiles
NQB = S // QB         # 4 q-blocks
NCORES = 8


def _round_tf32(x):
    u = np.ascontiguousarray(x, dtype=np.float32).view(np.uint32)
    r = (u + np.uint32(0x07FF) + ((u >> np.uint32(12)) & np.uint32(1))) & np.uint32(
        0xFFFFF000
    )
    return r.view(np.float32)


def _plan_from_mask(m):
    """m: [S, S] bool, True = masked (scores[q, k] masked).

    Returns (plans, patterns):
      plans[qb][kt] = None (skip) or (c0, nch, [(rel_chunk, uidx), ...])
        c0: first valid 128-chunk index within the q-block, nch: chunk count
      patterns: list of unique [128,128] float32 0/1 valid-masks (scoresT
        orientation: [k_partition, q_free]).
    """
    patterns = []
    pat_index = {}
    plans = []
    for qb in range(NQB):
        row = []
        for kt in range(NKT):
            # scoresT tile: partitions = k in [kt*128, ...), free = q chunk
            sub = m[qb * QB:(qb + 1) * QB, kt * CH:(kt + 1) * CH]  # [q, k]
            valid = (~sub).T  # [k, q] 128 x 512
            nchunks = QB // CH
            kinds = []
            for c in range(nchunks):
                ch = valid[:, c * CH:(c + 1) * CH]
                if ch.all():
                    kinds.append("full")
                elif not ch.any():
                    kinds.append("empty")
                else:
                    kinds.append("mixed")
            not_empty = [c for c in range(nchunks) if kinds[c] != "empty"]
            if not not_empty:
                row.append(None)
                continue
            c0, c1 = not_empty[0], not_empty[-1]
            mixed = []
            for c in range(c0, c1 + 1):
                if kinds[c] == "full":
                    continue
                pat = valid[:, c * CH:(c + 1) * CH].astype(np.float32)
                key = pat.tobytes()
                if key not in pat_index:
                    pat_index[key] = len(patterns)
                    patterns.append(pat)
                mixed.append((c - c0, pat_index[key]))
            row.append((c0, c1 - c0 + 1, mixed))
        plans.append(row)
    return plans, patterns


def _build(plans, n_patterns, guard_empty_rows):
    import concourse.bacc as bacc
    import concourse.tile as tile
    from concourse import mybir

    F32 = mybir.dt.float32
    F32R = mybir.dt.float32r
    AF = mybir.ActivationFunctionType

    nc = bacc.Bacc("TRN2", target_bir_lowering=False, debug=False)

    xq = nc.dram_tensor("xq_t", [D, S], F32R, kind="ExternalInput")
    xk = nc.dram_tensor("xk_t", [D, S], F32R, kind="ExternalInput")
    xv = nc.dram_tensor("xv_t", [D, S], F32R, kind="ExternalInput")
    wq = nc.dram_tensor("wq_t", [D, DL], F32R, kind="ExternalInput")
    wk = nc.dram_tensor("wk_t", [D, DL], F32R, kind="ExternalInput")
    wv = nc.dram_tensor("wv_t", [D, DL], F32R, kind="ExternalInput")
    wo = nc.dram_tensor("wo_t", [DL, D], F32R, kind="ExternalInput")
    bq = nc.dram_tensor("bq8", [1, DL], F32R, kind="ExternalInput")
    bk = nc.dram_tensor("bk", [1, DL], F32R, kind="ExternalInput")
    bv = nc.dram_tensor("bv", [1, DL], F32R, kind="ExternalInput")
    onesr = nc.dram_tensor("ones_row", [1, QB], F32R, kind="ExternalInput")
    onesc = nc.dram_tensor("ones_cols", [CH, NKT * HL], F32R, kind="ExternalInput")
    if n_patterns:
        maskp = nc.dram_tensor("maskp", [CH, n_patterns * CH], F32,
                               kind="ExternalInput")
    outT = nc.dram_tensor("outT", [D, S], F32, kind="ExternalOutput")

    MT = DL // CH      # 4: dq tiles / dcat tiles
    NST = S // QB      # 4 s-quarters

    with tile.TileContext(nc) as tc, ExitStack() as ctx:
        persist = ctx.enter_context(tc.tile_pool(name="persist", bufs=1))
        xin = ctx.enter_context(tc.tile_pool(name="xin", bufs=9))
        wt = ctx.enter_context(tc.tile_pool(name="wt", bufs=9))
        expp = ctx.enter_context(tc.tile_pool(name="expp", bufs=3))
        attp = ctx.enter_context(tc.tile_pool(name="attp", bufs=2))
        outp = ctx.enter_context(tc.tile_pool(name="outp", bufs=4))
        recp = ctx.enter_context(tc.tile_pool(name="recp", bufs=2))
        ps_mm = ctx.enter_context(tc.tile_pool(name="ps_mm", bufs=2, space="PSUM"))
        ps_sc = ctx.enter_context(tc.tile_pool(name="ps_sc", bufs=2, space="PSUM"))
        ps_raw = ctx.enter_context(tc.tile_pool(name="ps_raw", bufs=2, space="PSUM"))

        qt_all = persist.tile([CH, MT, S], F32R)       # Q.T  [dq, s]
        kt_all = persist.tile([CH, MT, S], F32R)       # K.T  [dk, s]
        v_all = persist.tile([CH, NKT, HL * (DK + 1)], F32R)  # V + ones cols
        wo_all = persist.tile([CH, MT, D], F32R)       # Wo_g.T
        ones_sb = persist.tile([1, QB], F32R)
        bq_sb = persist.tile([1, DL], F32R)
        bk_sb = persist.tile([1, DL], F32R)
        bv_sb = persist.tile([1, DL], F32R)
        if n_patterns:
            mp_sb = persist.tile([CH, n_patterns, CH], F32)
            nc.sync.dma_start(mp_sb[:], maskp.ap().rearrange(
                "p (u f) -> p u f", f=CH))

        nc.sync.dma_start(wo_all[:], wo.ap().rearrange("(t p) m -> p t m", p=CH))
        nc.sync.dma_start(ones_sb[:], onesr.ap())
        nc.sync.dma_start(bq_sb[:], bq.ap())
        nc.sync.dma_start(bk_sb[:], bk.ap())
        nc.sync.dma_start(bv_sb[:], bv.ap())
        # ones columns of v_all: [128, kt, h, 1] <- [128, kt*h]
        nc.sync.dma_start(
            v_all[:].rearrange("p s (h c) -> p s h c", c=DK + 1)[:, :, :, DK:DK + 1],
            onesc.ap().rearrange("p (s h o) -> p s h o", h=HL, o=1),
        )

        # ---------------- phase 1: projections ----------------
        NK = D // CH  # 8 contraction tiles

        def proj_qk(x_dram, w_dram, bias_sb, dst, scaleq):
            # dst[dq, s] = w.T.T @ x.T (+ bias row outer ones)
            w_tiles = []
            for kt in range(NK):
                wtile = wt.tile([CH, DL], F32R, tag="w")
                nc.sync.dma_start(wtile[:], w_dram.ap()[kt * CH:(kt + 1) * CH, :])
                w_tiles.append(wtile)
            for qu in range(NST):
                x_tiles = []
                for kt in range(NK):
                    xt = xin.tile([CH, QB], F32R, tag="x")
                    nc.sync.dma_start(
                        xt[:], x_dram.ap()[kt * CH:(kt + 1) * CH,
                                           qu * QB:(qu + 1) * QB])
                    x_tiles.append(xt)
                for m in range(MT):
                    ps = ps_mm.tile([CH, QB], F32, tag="mm")
                    for kt in range(NK):
                        nc.tensor.matmul(
                            ps[:], w_tiles[kt][:, m * CH:(m + 1) * CH],
                            x_tiles[kt][:], start=(kt == 0), stop=False)
                    nc.tensor.matmul(
                        ps[:], bias_sb[0:1, m * CH:(m + 1) * CH], ones_sb[0:1, :],
                        start=False, stop=True)
                    nc.vector.tensor_copy(
                        out=dst[:, m, qu * QB:(qu + 1) * QB], in_=ps[:])

        proj_qk(xq, wq, bq_sb, qt_all, True)
        proj_qk(xk, wk, bk_sb, kt_all, False)

        # V: v[s, dv] = x.T.T @ w.T (+ ones col bias)
        wv_tiles = []
        for kt in range(NK):
            wtile = wt.tile([CH, DL], F32R, tag="w")
            nc.sync.dma_start(wtile[:], wv.ap()[kt * CH:(kt + 1) * CH, :])
            wv_tiles.append(wtile)
        for qu in range(NST):
            x_tiles = []
            for kt in range(NK):
                xt = xin.tile([CH, QB], F32R, tag="x")
                nc.sync.dma_start(
                    xt[:], xv.ap()[kt * CH:(kt + 1) * CH, qu * QB:(qu + 1) * QB])
                x_tiles.append(xt)
            for j in range(QB // CH):
                st = qu * (QB // CH) + j
                ps = ps_mm.tile([CH, DL], F32, tag="mm")
                for kt in range(NK):
                    nc.tensor.matmul(
                        ps[:], x_tiles[kt][:, j * CH:(j + 1) * CH],
                        wv_tiles[kt][:], start=(kt == 0), stop=False)
                nc.tensor.matmul(
                    ps[:], ones_sb[0:1, 0:CH], bv_sb[0:1, :],
                    start=False, stop=True)
                nc.vector.tensor_copy(
                    out=v_all[:, st, :].rearrange(
                        "p (h c) -> p h c", c=DK + 1)[:, :, 0:DK],
                    in_=ps[:].rearrange("p (h c) -> p h c", c=DK),
                )

        # ---------------- phase 2+3: attention + out-proj per q-block ------
        for qb in range(NQB):
            att = attp.tile([CH, MT, QB], F32R, tag="att")
            kt_plan = [(kt, plans[qb][kt]) for kt in range(NKT)
                       if plans[qb][kt] is not None]
            # pair consecutive full-width clean tiles for wider ACT exp
            units = []
            i = 0
            while i < len(kt_plan):
                kt, (c0, nch, mixed) = kt_plan[i]
                full = (c0 == 0 and nch == QB // CH and not mixed)
                if full and i + 1 < len(kt_plan):
                    kt2, (c02, nch2, mixed2) = kt_plan[i + 1]
                    if c02 == 0 and nch2 == QB // CH and not mixed2:
                        units.append(("pair", kt, kt2))
                        i += 2
                        continue
                units.append(("single", kt, (c0, nch, mixed)))
                i += 1

            for h in range(HL):
                hp = (h % 2) * DK          # partition offset in qt/kt tiles
                hm = h // 2                # m-tile index
                raw = ps_raw.tile([DK + 1, QB], F32, tag="raw")
                first = True
                for unit in units:
                    if unit[0] == "pair":
                        _, kta, ktb = unit
                        sc = ps_sc.tile([CH, 2 * QB], F32, tag="sc")
                        for half, kt in ((0, kta), (1, ktb)):
                            nc.tensor.matmul(
                                sc[:, half * QB:(half + 1) * QB],
                                kt_all[hp:hp + DK, hm, kt * CH:(kt + 1) * CH],
                                qt_all[hp:hp + DK, hm, qb * QB:(qb + 1) * QB],
                                start=True, stop=True)
                        ex = expp.tile([CH, 2 * QB], F32R, tag="exp")
                        nc.scalar.activation(ex[:], sc[:], AF.Exp)
                        for half, kt in ((0, kta), (1, ktb)):
                            nc.tensor.matmul(
                                raw[:, :],
                                v_all[:, kt, h * (DK + 1):(h + 1) * (DK + 1)],
                                ex[:, half * QB:(half + 1) * QB],
                                start=first, stop=False,
                                skip_group_check=True)
                            first = False
                    else:
                        _, kt, (c0, nch, mixed) = unit
                        w = nch * CH
                        o = c0 * CH
                        sc = ps_sc.tile([CH, 2 * QB], F32, tag="sc")
                        nc.tensor.matmul(
                            sc[:, 0:w],
                            kt_all[hp:hp + DK, hm, kt * CH:(kt + 1) * CH],
                            qt_all[hp:hp + DK, hm,
                                   qb * QB + o:qb * QB + o + w],
                            start=True, stop=True)
                        ex = expp.tile([CH, 2 * QB], F32R, tag="exp")
                        nc.scalar.activation(ex[:, 0:w], sc[:, 0:w], AF.Exp)
                        for (rel, uidx) in mixed:
                            nc.vector.tensor_mul(
                                ex[:, rel * CH:(rel + 1) * CH],
                                ex[:, rel * CH:(rel + 1) * CH],
                                mp_sb[:, uidx, :])
                        nc.tensor.matmul(
                            raw[:, o:o + w],
                            v_all[:, kt, h * (DK + 1):(h + 1) * (DK + 1)],
                            ex[:, 0:w],
                            start=first, stop=False,
                            skip_group_check=True)
                        first = False
                # normalize -> attnT_cat
                rec = recp.tile([1, QB], F32, tag="rec")
                if guard_empty_rows:
                    den = recp.tile([1, QB], F32, tag="den")
                    nc.vector.tensor_scalar_max(den[:], raw[DK:DK + 1, :], 1e-30)
                    nc.vector.reciprocal(rec[:], den[:])
                else:
                    nc.vector.reciprocal(rec[:], raw[DK:DK + 1, :])
                recb = recp.tile([DK, QB], F32, tag="recb")
                nc.gpsimd.partition_broadcast(recb[:], rec[:])
                nc.vector.tensor_mul(
                    att[hp:hp + DK, hm, :], raw[0:DK, :], recb[:])

            # out-proj for this q-block
            for mo in range(D // CH):
                ps = ps_mm.tile([CH, QB], F32, tag="mm")
                for ct in range(MT):
                    nc.tensor.matmul(
                        ps[:], wo_all[:, ct, mo * CH:(mo + 1) * CH],
                        att[:, ct, :], start=(ct == 0), stop=(ct == MT - 1),
                        skip_group_check=True)
                ot = outp.tile([CH, QB], F32, tag="ot")
                nc.vector.tensor_copy(out=ot[:], in_=ps[:])
                nc.sync.dma_start(
                    outT.ap()[mo * CH:(mo + 1) * CH, qb * QB:(qb + 1) * QB],
                    ot[:])

    nc.compile()
    return nc


_CACHE = {}
LAST_RESULTS = None


def _install_ntff_shim():
    """Provide antenv.axon_hooks (NTFF profiling) when the image lacks it."""
    import sys, types, ctypes, contextlib
    if "antenv.axon_hooks" in sys.modules:
        return
    import antenv
    mod = types.ModuleType("antenv.axon_hooks")
    state = {"hook": None}
    mod.set_axon_ntff_profile_hook = lambda h: state.__setitem__("hook", h)
    mod.get_axon_ntff_profile_hook = lambda: state["hook"]
    sys.modules["antenv.axon_hooks"] = mod
    antenv.axon_hooks = mod
    try:
        lib = ctypes.CDLL("/opt/axon/libaxon_pjrt.so")
    except OSError:
        return
    if not hasattr(lib, "axon_start_nrt_profile"):
        return
    lib.axon_start_nrt_profile.argtypes = [
        ctypes.POINTER(ctypes.c_int64), ctypes.c_size_t]
    lib.axon_start_nrt_profile.restype = ctypes.c_int64
    lib.axon_stop_nrt_profile.argtypes = [ctypes.c_char_p]
    lib.axon_stop_nrt_profile.restype = ctypes.c_int64

    @contextlib.contextmanager
    def _hook(output_dir, device_ids):
        import jax
        jax.devices()
        if device_ids:
            ids = (ctypes.c_int64 * len(device_ids))(*device_ids)
            rc = lib.axon_start_nrt_profile(ids, len(device_ids))
        else:
            rc = lib.axon_start_nrt_profile(None, 0)
        if rc != 0:
            raise RuntimeError(f"axon_start_nrt_profile rc={rc}")
        try:
            yield
        finally:
            n = lib.axon_stop_nrt_profile(str(output_dir).encode())
            print(f"profile: {n} ntff file(s) in {output_dir}", file=sys.stderr)

    state["hook"] = _hook


def _get_nc(mask2d):
    key = hash(mask2d.tobytes())
    if key not in _CACHE:
        plans, patterns = _plan_from_mask(mask2d)
        # guard against fully-masked rows (reference maps softmax NaN -> 0)
        valid_any = (~mask2d).any(axis=1)
        guard = bool((~valid_any).any())
        _CACHE[key] = (_build(plans, len(patterns), guard), patterns)
    return _CACHE[key]


def kernel(query, key, value, mask, Wq, bq, Wk, bk, Wv, bv, Wo, bo):
    from concourse.bass_utils import run_bass_kernel_spmd

    query = np.asarray(query, dtype=np.float32)
    key_ = np.asarray(key, dtype=np.float32)
    value = np.asarray(value, dtype=np.float32)
    mask2d = np.asarray(mask, dtype=bool).reshape(S, S)
    Wq = np.asarray(Wq, dtype=np.float32)
    Wk = np.asarray(Wk, dtype=np.float32)
    Wv = np.asarray(Wv, dtype=np.float32)
    Wo = np.asarray(Wo, dtype=np.float32)
    bq = np.asarray(bq, dtype=np.float32)
    bk = np.asarray(bk, dtype=np.float32)
    bv = np.asarray(bv, dtype=np.float32)
    bo = np.asarray(bo, dtype=np.float32)

    nc, patterns = _get_nc(mask2d)

    n_pat = len(patterns)
    if n_pat:
        mp = np.empty((CH, n_pat * CH), np.float32)
        for u, pat in enumerate(patterns):
            mp[:, u * CH:(u + 1) * CH] = pat
    ones_row = np.ones((1, QB), np.float32)
    ones_cols = np.ones((CH, NKT * HL), np.float32)

    in_maps = []
    for c in range(NCORES):
        b, g = divmod(c, 2)
        gsl = slice(DL * g, DL * (g + 1))
        m = {
            "xq_t": _round_tf32(query[b].T),
            "xk_t": _round_tf32(key_[b].T),
            "xv_t": _round_tf32(value[b].T),
            "wq_t": _round_tf32(Wq[gsl].T * 0.125),
            "wk_t": _round_tf32(Wk[gsl].T),
            "wv_t": _round_tf32(Wv[gsl].T),
            "wo_t": _round_tf32(Wo[:, gsl].T),
            "bq8": _round_tf32(bq[gsl].reshape(1, DL) * 0.125),
            "bk": _round_tf32(bk[gsl].reshape(1, DL)),
            "bv": _round_tf32(bv[gsl].reshape(1, DL)),
            "ones_row": ones_row,
            "ones_cols": ones_cols,
        }
        if n_pat:
            m["maskp"] = mp
        in_maps.append(m)

    import os
    kwargs = {}
    if os.environ.get("BASS_MHA_TRACE"):
        _install_ntff_shim()
        kwargs = dict(trace=True, trace_cores=[0])
    res = run_bass_kernel_spmd(nc, in_maps, core_ids=list(range(NCORES)), **kwargs)
    global LAST_RESULTS
    LAST_RESULTS = res

    out = np.empty((B, S, D), np.float32)
    for b in range(B):
        acc = res.results[2 * b]["outT"] + res.results[2 * b + 1]["outT"]
        out[b] = acc.T + bo[None, :]
    return out


# revision 8
# speedup vs baseline: 1.3732x; 1.3732x over previous
"""Multi-head attention (B=4, S=2048, D=1024, H=16) on 8 TRN2 NeuronCores.

Sharding: core c handles batch b=c//2 and head-group g=c%2 (8 heads, 512 of
the 1024 model dims).  Wq/Wk/Wv column-parallel, Wo row-parallel; the two
head-group partial outputs per batch are summed on the host (no collectives).

Per-core dataflow (all matmuls in f32r = TF32, fp32 PSUM accumulate):
  phase 1: Q.T = (Wq/8) @ x.T   [512,2048]   (bias via K=1 ones-row matmul)
           K.T = Wk @ x.T       [512,2048]
           V   = x @ Wv.T       [2048,512]   stored head-interleaved with a
                                             ones column: [128,16,8*65]
  phase 2 (per 512-wide q-block, per head):
           scoresT[k,q] = K_h.T.T @ Q_h.T    (K=64 contraction)
           expT = exp(scoresT) on ACT (no max subtraction needed: |s|<~10)
           mixed 128-chunks multiplied by 0/1 mask tiles; fully-masked
           column windows simply not computed (causal skipping)
           raw[65,512] += [V_h|1].T @ expT   (row 64 = softmax denominator)
           attnT[h] = raw[0:64] * (1/raw[64]) broadcast (gpsimd bcast + DVE)
  phase 3 (per q-block): outT += Wo_g.T.T @ attnT_cat -> [1024,2048] partial
Host: out[b] = (partial_g0 + partial_g1).T + bo
"""

import numpy as np
import ml_dtypes
from contextlib import ExitStack

B = 4
S = 2048
D = 1024
H = 16
DK = 64
G = 2                 # head groups
HL = H // G           # heads per core = 8
DL = D // G           # local head dims = 512
QB = 512              # q-block width
CH = 128              # chunk / k-tile width
NKT = S // CH         # 16 k-tiles
NQB = S // QB         # 4 q-blocks
NCORES = 8


def _bf16(x):
    return np.ascontiguousarray(x, dtype=np.float32).astype(ml_dtypes.bfloat16)


def _plan_from_mask(m):
    """m: [S, S] bool, True = masked (scores[q, k] masked).

    Returns (plans, patterns):
      plans[qb][kt] = None (skip) or (c0, nch, [(rel_chunk, uidx), ...])
        c0: first valid 128-chunk index within the q-block, nch: chunk count
      patterns: list of unique [128,128] float32 0/1 valid-masks (scoresT
        orientation: [k_partition, q_free]).
    """
    patterns = []
    pat_index = {}
    plans = []
    for qb in range(NQB):
        row = []
        for kt in range(NKT):
            # scoresT tile: partitions = k in [kt*128, ...), free = q chunk
            sub = m[qb * QB:(qb + 1) * QB, kt * CH:(kt + 1) * CH]  # [q, k]
            valid = (~sub).T  # [k, q] 128 x 512
            nchunks = QB // CH
            kinds = []
            for c in range(nchunks):
                ch = valid[:, c * CH:(c + 1) * CH]
                if ch.all():
                    kinds.append("full")
                elif not ch.any():
                    kinds.append("empty")
                else:
                    kinds.append("mixed")
            not_empty = [c for c in range(nchunks) if kinds[c] != "empty"]
            if not not_empty:
                row.append(None)
                continue
            c0, c1 = not_empty[0], not_empty[-1]
            mixed = []
            for c in range(c0, c1 + 1):
                if kinds[c] == "full":
                    continue
                pat = valid[:, c * CH:(c + 1) * CH].astype(np.float32)
                key = pat.tobytes()
                if key not in pat_index:
                    pat_index[key] = len(patterns)
                    patterns.append(pat)
                mixed.append((c - c0, pat_index[key]))
            row.append((c0, c1 - c0 + 1, mixed))
        plans.append(row)
    return plans, patterns


def _build(plans, n_patterns, guard_empty_rows):
    import concourse.bacc as bacc
    import concourse.tile as tile
    from concourse import mybir

    F32 = mybir.dt.float32
    BF16 = mybir.dt.bfloat16
    AF = mybir.ActivationFunctionType

    nc = bacc.Bacc("TRN2", target_bir_lowering=False, debug=False)

    xq = nc.dram_tensor("xq_t", [D, S], BF16, kind="ExternalInput")
    xk = nc.dram_tensor("xk_t", [D, S], BF16, kind="ExternalInput")
    xv = nc.dram_tensor("xv_t", [D, S], BF16, kind="ExternalInput")
    wq = nc.dram_tensor("wq_t", [D, DL], BF16, kind="ExternalInput")
    wk = nc.dram_tensor("wk_t", [D, DL], BF16, kind="ExternalInput")
    wv = nc.dram_tensor("wv_t", [D, DL], BF16, kind="ExternalInput")
    wo = nc.dram_tensor("wo_t", [DL, D], BF16, kind="ExternalInput")
    bq = nc.dram_tensor("bq8", [1, DL], BF16, kind="ExternalInput")
    bk = nc.dram_tensor("bk", [1, DL], BF16, kind="ExternalInput")
    bv = nc.dram_tensor("bv", [1, DL], BF16, kind="ExternalInput")
    onesr = nc.dram_tensor("ones_row", [1, QB], BF16, kind="ExternalInput")
    onesc = nc.dram_tensor("ones_cols", [CH, NKT * HL], BF16, kind="ExternalInput")
    if n_patterns:
        maskp = nc.dram_tensor("maskp", [CH, n_patterns * CH], BF16,
                               kind="ExternalInput")
    outT = nc.dram_tensor("outT", [D, S], F32, kind="ExternalOutput")

    MT = DL // CH      # 4: dq tiles / dcat tiles
    NST = S // QB      # 4 s-quarters

    with tile.TileContext(nc) as tc, ExitStack() as ctx:
        persist = ctx.enter_context(tc.tile_pool(name="persist", bufs=1))
        xin = ctx.enter_context(tc.tile_pool(name="xin", bufs=9))
        wt = ctx.enter_context(tc.tile_pool(name="wt", bufs=9))
        expp = ctx.enter_context(tc.tile_pool(name="expp", bufs=3))
        attp = ctx.enter_context(tc.tile_pool(name="attp", bufs=2))
        outp = ctx.enter_context(tc.tile_pool(name="outp", bufs=4))
        recp = ctx.enter_context(tc.tile_pool(name="recp", bufs=2))
        ps_mm = ctx.enter_context(tc.tile_pool(name="ps_mm", bufs=2, space="PSUM"))
        ps_sc = ctx.enter_context(tc.tile_pool(name="ps_sc", bufs=2, space="PSUM"))
        ps_raw = ctx.enter_context(tc.tile_pool(name="ps_raw", bufs=2, space="PSUM"))

        qt_all = persist.tile([CH, MT, S], BF16)       # Q.T  [dq, s]
        kt_all = persist.tile([CH, MT, S], BF16)       # K.T  [dk, s]
        v_all = persist.tile([CH, NKT, HL * (DK + 1)], BF16)  # V + ones cols
        wo_all = persist.tile([CH, MT, D], BF16)       # Wo_g.T
        ones_sb = persist.tile([1, QB], BF16)
        bq_sb = persist.tile([1, DL], BF16)
        bk_sb = persist.tile([1, DL], BF16)
        bv_sb = persist.tile([1, DL], BF16)
        if n_patterns:
            mp_sb = persist.tile([CH, n_patterns, CH], BF16)
            nc.sync.dma_start(mp_sb[:], maskp.ap().rearrange(
                "p (u f) -> p u f", f=CH))

        nc.sync.dma_start(wo_all[:], wo.ap().rearrange("(t p) m -> p t m", p=CH))
        nc.sync.dma_start(ones_sb[:], onesr.ap())
        nc.sync.dma_start(bq_sb[:], bq.ap())
        nc.sync.dma_start(bk_sb[:], bk.ap())
        nc.sync.dma_start(bv_sb[:], bv.ap())
        # ones columns of v_all: [128, kt, h, 1] <- [128, kt*h]
        nc.sync.dma_start(
            v_all[:].rearrange("p s (h c) -> p s h c", c=DK + 1)[:, :, :, DK:DK + 1],
            onesc.ap().rearrange("p (s h o) -> p s h o", h=HL, o=1),
        )

        # ---------------- phase 1: projections ----------------
        NK = D // CH  # 8 contraction tiles

        def proj_qk(x_dram, w_dram, bias_sb, dst, scaleq):
            # dst[dq, s] = w.T.T @ x.T (+ bias row outer ones)
            w_tiles = []
            for kt in range(NK):
                wtile = wt.tile([CH, DL], BF16, tag="w")
                nc.sync.dma_start(wtile[:], w_dram.ap()[kt * CH:(kt + 1) * CH, :])
                w_tiles.append(wtile)
            for qu in range(NST):
                x_tiles = []
                for kt in range(NK):
                    xt = xin.tile([CH, QB], BF16, tag="x")
                    nc.sync.dma_start(
                        xt[:], x_dram.ap()[kt * CH:(kt + 1) * CH,
                                           qu * QB:(qu + 1) * QB])
                    x_tiles.append(xt)
                for m in range(MT):
                    ps = ps_mm.tile([CH, QB], F32, tag="mm")
                    for kt in range(NK):
                        nc.tensor.matmul(
                            ps[:], w_tiles[kt][:, m * CH:(m + 1) * CH],
                            x_tiles[kt][:], start=(kt == 0), stop=False)
                    nc.tensor.matmul(
                        ps[:], bias_sb[0:1, m * CH:(m + 1) * CH], ones_sb[0:1, :],
                        start=False, stop=True)
                    nc.vector.tensor_copy(
                        out=dst[:, m, qu * QB:(qu + 1) * QB], in_=ps[:])

        proj_qk(xq, wq, bq_sb, qt_all, True)
        proj_qk(xk, wk, bk_sb, kt_all, False)

        # V: v[s, dv] = x.T.T @ w.T (+ ones col bias)
        wv_tiles = []
        for kt in range(NK):
            wtile = wt.tile([CH, DL], BF16, tag="w")
            nc.sync.dma_start(wtile[:], wv.ap()[kt * CH:(kt + 1) * CH, :])
            wv_tiles.append(wtile)
        for qu in range(NST):
            x_tiles = []
            for kt in range(NK):
                xt = xin.tile([CH, QB], BF16, tag="x")
                nc.sync.dma_start(
                    xt[:], xv.ap()[kt * CH:(kt + 1) * CH, qu * QB:(qu + 1) * QB])
                x_tiles.append(xt)
            for j in range(QB // CH):
                st = qu * (QB // CH) + j
                ps = ps_mm.tile([CH, DL], F32, tag="mm")
                for kt in range(NK):
                    nc.tensor.matmul(
                        ps[:], x_tiles[kt][:, j * CH:(j + 1) * CH],
                        wv_tiles[kt][:], start=(kt == 0), stop=False)
                nc.tensor.matmul(
                    ps[:], ones_sb[0:1, 0:CH], bv_sb[0:1, :],
                    start=False, stop=True)
                nc.vector.tensor_copy(
                    out=v_all[:, st, :].rearrange(
                        "p (h c) -> p h c", c=DK + 1)[:, :, 0:DK],
                    in_=ps[:].rearrange("p (h c) -> p h c", c=DK),
                )

        # ---------------- phase 2+3: attention + out-proj per q-block ------
        for qb in range(NQB):
            att = attp.tile([CH, MT, QB], BF16, tag="att")
            kt_plan = [(kt, plans[qb][kt]) for kt in range(NKT)
                       if plans[qb][kt] is not None]
            # pair consecutive full-width clean tiles for wider ACT exp
            units = []
            i = 0
            while i < len(kt_plan):
                kt, (c0, nch, mixed) = kt_plan[i]
                full = (c0 == 0 and nch == QB // CH and not mixed)
                if full and i + 1 < len(kt_plan):
                    kt2, (c02, nch2, mixed2) = kt_plan[i + 1]
                    if c02 == 0 and nch2 == QB // CH and not mixed2:
                        units.append(("pair", kt, kt2))
                        i += 2
                        continue
                units.append(("single", kt, (c0, nch, mixed)))
                i += 1

            for h in range(HL):
                hp = (h % 2) * DK          # partition offset in qt/kt tiles
                hm = h // 2                # m-tile index
                raw = ps_raw.tile([DK + 1, QB], F32, tag="raw")
                first = True
                for unit in units:
                    if unit[0] == "pair":
                        _, kta, ktb = unit
                        sc = ps_sc.tile([CH, 2 * QB], F32, tag="sc")
                        for half, kt in ((0, kta), (1, ktb)):
                            nc.tensor.matmul(
                                sc[:, half * QB:(half + 1) * QB],
                                kt_all[hp:hp + DK, hm, kt * CH:(kt + 1) * CH],
                                qt_all[hp:hp + DK, hm, qb * QB:(qb + 1) * QB],
                                start=True, stop=True)
                        ex = expp.tile([CH, 2 * QB], BF16, tag="exp")
                        nc.scalar.activation(ex[:], sc[:], AF.Exp)
                        for half, kt in ((0, kta), (1, ktb)):
                            nc.tensor.matmul(
                                raw[:, :],
                                v_all[:, kt, h * (DK + 1):(h + 1) * (DK + 1)],
                                ex[:, half * QB:(half + 1) * QB],
                                start=first, stop=False,
                                skip_group_check=True)
                            first = False
                    else:
                        _, kt, (c0, nch, mixed) = unit
                        w = nch * CH
                        o = c0 * CH
                        sc = ps_sc.tile([CH, 2 * QB], F32, tag="sc")
                        nc.tensor.matmul(
                            sc[:, 0:w],
                            kt_all[hp:hp + DK, hm, kt * CH:(kt + 1) * CH],
                            qt_all[hp:hp + DK, hm,
                                   qb * QB + o:qb * QB + o + w],
                            start=True, stop=True)
                        ex = expp.tile([CH, 2 * QB], BF16, tag="exp")
                        nc.scalar.activation(ex[:, 0:w], sc[:, 0:w], AF.Exp)
                        for (rel, uidx) in mixed:
                            nc.vector.tensor_mul(
                                ex[:, rel * CH:(rel + 1) * CH],
                                ex[:, rel * CH:(rel + 1) * CH],
                                mp_sb[:, uidx, :])
                        nc.tensor.matmul(
                            raw[:, o:o + w],
                            v_all[:, kt, h * (DK + 1):(h + 1) * (DK + 1)],
                            ex[:, 0:w],
                            start=first, stop=False,
                            skip_group_check=True)
                        first = False
                # normalize -> attnT_cat
                rec = recp.tile([1, QB], F32, tag="rec")
                scr = recp.tile([1, QB], F32, tag="scr")
                den = recp.tile([1, QB], F32, tag="den")
                if guard_empty_rows:
                    nc.vector.tensor_scalar_max(den[:], raw[DK:DK + 1, :], 1e-30)
                else:
                    nc.vector.tensor_copy(den[:], raw[DK:DK + 1, :])
                nc.vector.reciprocal_approx_accurate(
                    out=rec[:], in_=den[:], scratch=scr[:])
                recb = recp.tile([DK, QB], F32, tag="recb")
                nc.gpsimd.partition_broadcast(recb[:], rec[:])
                nc.vector.tensor_mul(
                    att[hp:hp + DK, hm, :], raw[0:DK, :], recb[:])

            # out-proj for this q-block
            for mo in range(D // CH):
                ps = ps_mm.tile([CH, QB], F32, tag="mm")
                for ct in range(MT):
                    nc.tensor.matmul(
                        ps[:], wo_all[:, ct, mo * CH:(mo + 1) * CH],
                        att[:, ct, :], start=(ct == 0), stop=(ct == MT - 1),
                        skip_group_check=True)
                ot = outp.tile([CH, QB], F32, tag="ot")
                nc.vector.tensor_copy(out=ot[:], in_=ps[:])
                nc.sync.dma_start(
                    outT.ap()[mo * CH:(mo + 1) * CH, qb * QB:(qb + 1) * QB],
                    ot[:])

    nc.compile()
    return nc


_CACHE = {}
LAST_RESULTS = None


def _install_ntff_shim():
    """Provide antenv.axon_hooks (NTFF profiling) when the image lacks it."""
    import sys, types, ctypes, contextlib
    if "antenv.axon_hooks" in sys.modules:
        return
    import antenv
    mod = types.ModuleType("antenv.axon_hooks")
    state = {"hook": None}
    mod.set_axon_ntff_profile_hook = lambda h: state.__setitem__("hook", h)
    mod.get_axon_ntff_profile_hook = lambda: state["hook"]
    sys.modules["antenv.axon_hooks"] = mod
    antenv.axon_hooks = mod
    try:
        lib = ctypes.CDLL("/opt/axon/libaxon_pjrt.so")
    except OSError:
        return
    if not hasattr(lib, "axon_start_nrt_profile"):
        return
    lib.axon_start_nrt_profile.argtypes = [
        ctypes.POINTER(ctypes.c_int64), ctypes.c_size_t]
    lib.axon_start_nrt_profile.restype = ctypes.c_int64
    lib.axon_stop_nrt_profile.argtypes = [ctypes.c_char_p]
    lib.axon_stop_nrt_profile.restype = ctypes.c_int64

    @contextlib.contextmanager
    def _hook(output_dir, device_ids):
        import jax
        jax.devices()
        if device_ids:
            ids = (ctypes.c_int64 * len(device_ids))(*device_ids)
            rc = lib.axon_start_nrt_profile(ids, len(device_ids))
        else:
            rc = lib.axon_start_nrt_profile(None, 0)
        if rc != 0:
            raise RuntimeError(f"axon_start_nrt_profile rc={rc}")
        try:
            yield
        finally:
            n = lib.axon_stop_nrt_profile(str(output_dir).encode())
            print(f"profile: {n} ntff file(s) in {output_dir}", file=sys.stderr)

    state["hook"] = _hook


def _get_nc(mask2d):
    key = hash(mask2d.tobytes())
    if key not in _CACHE:
        plans, patterns = _plan_from_mask(mask2d)
        # guard against fully-masked rows (reference maps softmax NaN -> 0)
        valid_any = (~mask2d).any(axis=1)
        guard = bool((~valid_any).any())
        _CACHE[key] = (_build(plans, len(patterns), guard), patterns)
    return _CACHE[key]


def kernel(query, key, value, mask, Wq, bq, Wk, bk, Wv, bv, Wo, bo):
    from concourse.bass_utils import run_bass_kernel_spmd

    query = np.asarray(query, dtype=np.float32)
    key_ = np.asarray(key, dtype=np.float32)
    value = np.asarray(value, dtype=np.float32)
    mask2d = np.asarray(mask, dtype=bool).reshape(S, S)
    Wq = np.asarray(Wq, dtype=np.float32)
    Wk = np.asarray(Wk, dtype=np.float32)
    Wv = np.asarray(Wv, dtype=np.float32)
    Wo = np.asarray(Wo, dtype=np.float32)
    bq = np.asarray(bq, dtype=np.float32)
    bk = np.asarray(bk, dtype=np.float32)
    bv = np.asarray(bv, dtype=np.float32)
    bo = np.asarray(bo, dtype=np.float32)

    nc, patterns = _get_nc(mask2d)

    n_pat = len(patterns)
    if n_pat:
        mp = np.empty((CH, n_pat * CH), np.float32)
        for u, pat in enumerate(patterns):
            mp[:, u * CH:(u + 1) * CH] = pat
        mp = mp.astype(ml_dtypes.bfloat16)
    ones_row = np.ones((1, QB), ml_dtypes.bfloat16)
    ones_cols = np.ones((CH, NKT * HL), ml_dtypes.bfloat16)

    in_maps = []
    for c in range(NCORES):
        b, g = divmod(c, 2)
        gsl = slice(DL * g, DL * (g + 1))
        m = {
            "xq_t": _bf16(query[b].T),
            "xk_t": _bf16(key_[b].T),
            "xv_t": _bf16(value[b].T),
            "wq_t": _bf16(Wq[gsl].T * 0.125),
            "wk_t": _bf16(Wk[gsl].T),
            "wv_t": _bf16(Wv[gsl].T),
            "wo_t": _bf16(Wo[:, gsl].T),
            "bq8": _bf16(bq[gsl].reshape(1, DL) * 0.125),
            "bk": _bf16(bk[gsl].reshape(1, DL)),
            "bv": _bf16(bv[gsl].reshape(1, DL)),
            "ones_row": ones_row,
            "ones_cols": ones_cols,
        }
        if n_pat:
            m["maskp"] = mp
        in_maps.append(m)

    import os
    kwargs = {}
    if os.environ.get("BASS_MHA_TRACE"):
        _install_ntff_shim()
        kwargs = dict(trace=True, trace_cores=[0])
    res = run_bass_kernel_spmd(nc, in_maps, core_ids=list(range(NCORES)), **kwargs)
    global LAST_RESULTS
    LAST_RESULTS = res

    out = np.empty((B, S, D), np.float32)
    for b in range(B):
        acc = res.results[2 * b]["outT"] + res.results[2 * b + 1]["outT"]
        out[b] = acc.T + bo[None, :]
    return out


# revision 9
# speedup vs baseline: 1.5603x; 1.1363x over previous
"""Multi-head attention (B=4, S=2048, D=1024, H=16) on 8 TRN2 NeuronCores.

Sharding: core c handles batch b=c//2 and head-group g=c%2 (8 heads, 512 of
the 1024 model dims).  Wq/Wk/Wv column-parallel, Wo row-parallel; the two
head-group partial outputs per batch are summed on the host (no collectives).

Per-core dataflow (all matmuls in f32r = TF32, fp32 PSUM accumulate):
  phase 1: Q.T = (Wq/8) @ x.T   [512,2048]   (bias via K=1 ones-row matmul)
           K.T = Wk @ x.T       [512,2048]
           V   = x @ Wv.T       [2048,512]   stored head-interleaved with a
                                             ones column: [128,16,8*65]
  phase 2 (per 512-wide q-block, per head):
           scoresT[k,q] = K_h.T.T @ Q_h.T    (K=64 contraction)
           expT = exp(scoresT) on ACT (no max subtraction needed: |s|<~10)
           mixed 128-chunks multiplied by 0/1 mask tiles; fully-masked
           column windows simply not computed (causal skipping)
           raw[65,512] += [V_h|1].T @ expT   (row 64 = softmax denominator)
           attnT[h] = raw[0:64] * (1/raw[64]) broadcast (gpsimd bcast + DVE)
  phase 3 (per q-block): outT += Wo_g.T.T @ attnT_cat -> [1024,2048] partial
Host: out[b] = (partial_g0 + partial_g1).T + bo
"""

import numpy as np
import ml_dtypes
from contextlib import ExitStack

B = 4
S = 2048
D = 1024
H = 16
DK = 64
G = 2                 # head groups
HL = H // G           # heads per core = 8
DL = D // G           # local head dims = 512
QB = 512              # q-block width
CH = 128              # chunk / k-tile width
NKT = S // CH         # 16 k-tiles
NQB = S // QB         # 4 q-blocks
NCORES = 8


def _bf16(x):
    return np.ascontiguousarray(x, dtype=np.float32).astype(ml_dtypes.bfloat16)


def _plan_from_mask(m):
    """m: [S, S] bool, True = masked (scores[q, k] masked).

    Returns (plans, patterns):
      plans[qb][kt] = None (skip) or (c0, nch, [(rel_chunk, uidx), ...])
        c0: first valid 128-chunk index within the q-block, nch: chunk count
      patterns: list of unique [128,128] float32 0/1 valid-masks (scoresT
        orientation: [k_partition, q_free]).
    """
    patterns = []
    pat_index = {}
    plans = []
    for qb in range(NQB):
        row = []
        for kt in range(NKT):
            # scoresT tile: partitions = k in [kt*128, ...), free = q chunk
            sub = m[qb * QB:(qb + 1) * QB, kt * CH:(kt + 1) * CH]  # [q, k]
            valid = (~sub).T  # [k, q] 128 x 512
            nchunks = QB // CH
            kinds = []
            for c in range(nchunks):
                ch = valid[:, c * CH:(c + 1) * CH]
                if ch.all():
                    kinds.append("full")
                elif not ch.any():
                    kinds.append("empty")
                else:
                    kinds.append("mixed")
            not_empty = [c for c in range(nchunks) if kinds[c] != "empty"]
            if not not_empty:
                row.append(None)
                continue
            c0, c1 = not_empty[0], not_empty[-1]
            mixed = []
            for c in range(c0, c1 + 1):
                if kinds[c] == "full":
                    continue
                pat = valid[:, c * CH:(c + 1) * CH].astype(np.float32)
                key = pat.tobytes()
                if key not in pat_index:
                    pat_index[key] = len(patterns)
                    patterns.append(pat)
                mixed.append((c - c0, pat_index[key]))
            row.append((c0, c1 - c0 + 1, mixed))
        plans.append(row)
    return plans, patterns


def _build(plans, n_patterns, guard_empty_rows):
    import concourse.bacc as bacc
    import concourse.tile as tile
    from concourse import mybir

    F32 = mybir.dt.float32
    BF16 = mybir.dt.bfloat16
    AF = mybir.ActivationFunctionType

    nc = bacc.Bacc("TRN2", target_bir_lowering=False, debug=False)

    xq = nc.dram_tensor("xq_t", [D, S], BF16, kind="ExternalInput")
    xk = nc.dram_tensor("xk_t", [D, S], BF16, kind="ExternalInput")
    xv = nc.dram_tensor("xv_t", [D, S], BF16, kind="ExternalInput")
    wq = nc.dram_tensor("wq_t", [D, DL], BF16, kind="ExternalInput")
    wk = nc.dram_tensor("wk_t", [D, DL], BF16, kind="ExternalInput")
    wv = nc.dram_tensor("wv_t", [D, DL], BF16, kind="ExternalInput")
    wo = nc.dram_tensor("wo_t", [DL, D], BF16, kind="ExternalInput")
    bq = nc.dram_tensor("bq8", [1, DL], BF16, kind="ExternalInput")
    bk = nc.dram_tensor("bk", [1, DL], BF16, kind="ExternalInput")
    bv = nc.dram_tensor("bv", [1, DL], BF16, kind="ExternalInput")
    onesr = nc.dram_tensor("ones_row", [1, QB], BF16, kind="ExternalInput")
    onesc = nc.dram_tensor("ones_cols", [CH, NKT * HL], BF16, kind="ExternalInput")
    if n_patterns:
        maskp = nc.dram_tensor("maskp", [CH, n_patterns * CH], BF16,
                               kind="ExternalInput")
    outT = nc.dram_tensor("outT", [D, S], F32, kind="ExternalOutput")

    MT = DL // CH      # 4: dq tiles / dcat tiles
    NST = S // QB      # 4 s-quarters

    with tile.TileContext(nc) as tc, ExitStack() as ctx:
        persist = ctx.enter_context(tc.tile_pool(name="persist", bufs=1))
        xin = ctx.enter_context(tc.tile_pool(name="xin", bufs=16))
        wt = ctx.enter_context(tc.tile_pool(name="wt", bufs=25))
        expp = ctx.enter_context(tc.tile_pool(name="expp", bufs=6))
        attp = ctx.enter_context(tc.tile_pool(name="attp", bufs=2))
        outp = ctx.enter_context(tc.tile_pool(name="outp", bufs=4))
        recp = ctx.enter_context(tc.tile_pool(name="recp", bufs=2))
        ps_mm = ctx.enter_context(tc.tile_pool(name="ps_mm", bufs=2, space="PSUM"))
        ps_sc = ctx.enter_context(tc.tile_pool(name="ps_sc", bufs=4, space="PSUM"))
        ps_raw = ctx.enter_context(tc.tile_pool(name="ps_raw", bufs=2, space="PSUM"))

        qt_all = persist.tile([CH, MT, S], BF16)       # Q.T  [dq, s]
        kt_all = persist.tile([CH, MT, S], BF16)       # K.T  [dk, s]
        v_all = persist.tile([CH, NKT, HL * (DK + 1)], BF16)  # V + ones cols
        wo_all = persist.tile([CH, MT, D], BF16)       # Wo_g.T
        ones_sb = persist.tile([1, QB], BF16)
        bq_sb = persist.tile([1, DL], BF16)
        bk_sb = persist.tile([1, DL], BF16)
        bv_sb = persist.tile([1, DL], BF16)
        if n_patterns:
            mp_sb = persist.tile([CH, n_patterns, CH], BF16)
            nc.sync.dma_start(mp_sb[:], maskp.ap().rearrange(
                "p (u f) -> p u f", f=CH))

        nc.sync.dma_start(wo_all[:], wo.ap().rearrange("(t p) m -> p t m", p=CH))
        nc.sync.dma_start(ones_sb[:], onesr.ap())
        nc.sync.dma_start(bq_sb[:], bq.ap())
        nc.sync.dma_start(bk_sb[:], bk.ap())
        nc.sync.dma_start(bv_sb[:], bv.ap())
        # ones columns of v_all: [128, kt, h, 1] <- [128, kt*h]
        nc.sync.dma_start(
            v_all[:].rearrange("p s (h c) -> p s h c", c=DK + 1)[:, :, :, DK:DK + 1],
            onesc.ap().rearrange("p (s h o) -> p s h o", h=HL, o=1),
        )

        # ---------------- phase 1: projections ----------------
        NK = D // CH  # 8 contraction tiles

        def proj_qk(x_dram, w_dram, bias_sb, dst, scaleq):
            # dst[dq, s] = w.T.T @ x.T (+ bias row outer ones)
            w_tiles = []
            for kt in range(NK):
                wtile = wt.tile([CH, DL], BF16, tag="w")
                nc.sync.dma_start(wtile[:], w_dram.ap()[kt * CH:(kt + 1) * CH, :])
                w_tiles.append(wtile)
            for qu in range(NST):
                x_tiles = []
                for kt in range(NK):
                    xt = xin.tile([CH, QB], BF16, tag="x")
                    nc.sync.dma_start(
                        xt[:], x_dram.ap()[kt * CH:(kt + 1) * CH,
                                           qu * QB:(qu + 1) * QB])
                    x_tiles.append(xt)
                for m in range(MT):
                    ps = ps_mm.tile([CH, QB], F32, tag="mm")
                    for kt in range(NK):
                        nc.tensor.matmul(
                            ps[:], w_tiles[kt][:, m * CH:(m + 1) * CH],
                            x_tiles[kt][:], start=(kt == 0), stop=False)
                    nc.tensor.matmul(
                        ps[:], bias_sb[0:1, m * CH:(m + 1) * CH], ones_sb[0:1, :],
                        start=False, stop=True)
                    nc.vector.tensor_copy(
                        out=dst[:, m, qu * QB:(qu + 1) * QB], in_=ps[:])

        proj_qk(xq, wq, bq_sb, qt_all, True)
        proj_qk(xk, wk, bk_sb, kt_all, False)

        # V: v[s, dv] = x.T.T @ w.T (+ ones col bias)
        wv_tiles = []
        for kt in range(NK):
            wtile = wt.tile([CH, DL], BF16, tag="w")
            nc.sync.dma_start(wtile[:], wv.ap()[kt * CH:(kt + 1) * CH, :])
            wv_tiles.append(wtile)
        for qu in range(NST):
            x_tiles = []
            for kt in range(NK):
                xt = xin.tile([CH, QB], BF16, tag="x")
                nc.sync.dma_start(
                    xt[:], xv.ap()[kt * CH:(kt + 1) * CH, qu * QB:(qu + 1) * QB])
                x_tiles.append(xt)
            for j in range(QB // CH):
                st = qu * (QB // CH) + j
                ps = ps_mm.tile([CH, DL], F32, tag="mm")
                for kt in range(NK):
                    nc.tensor.matmul(
                        ps[:], x_tiles[kt][:, j * CH:(j + 1) * CH],
                        wv_tiles[kt][:], start=(kt == 0), stop=False)
                nc.tensor.matmul(
                    ps[:], ones_sb[0:1, 0:CH], bv_sb[0:1, :],
                    start=False, stop=True)
                nc.vector.tensor_copy(
                    out=v_all[:, st, :].rearrange(
                        "p (h c) -> p h c", c=DK + 1)[:, :, 0:DK],
                    in_=ps[:].rearrange("p (h c) -> p h c", c=DK),
                )

        # ---------------- phase 2+3: attention + out-proj per q-block ------
        for qb in range(NQB):
            att = attp.tile([CH, MT, QB], BF16, tag="att")
            kt_plan = [(kt, plans[qb][kt]) for kt in range(NKT)
                       if plans[qb][kt] is not None]

            for h in range(HL):
                hp = (h % 2) * DK          # partition offset in qt/kt tiles
                hm = h // 2                # m-tile index
                raw = ps_raw.tile([DK + 1, QB], F32, tag="raw")
                first = True
                for kt, (c0, nch, mixed) in kt_plan:
                    w = nch * CH
                    o = c0 * CH
                    sc = ps_sc.tile([CH, QB], F32, tag="sc")
                    nc.tensor.matmul(
                        sc[:, 0:w],
                        kt_all[hp:hp + DK, hm, kt * CH:(kt + 1) * CH],
                        qt_all[hp:hp + DK, hm,
                               qb * QB + o:qb * QB + o + w],
                        start=True, stop=True)
                    ex = expp.tile([CH, QB], BF16, tag="exp")
                    nc.scalar.activation(ex[:, 0:w], sc[:, 0:w], AF.Exp)
                    for (rel, uidx) in mixed:
                        nc.vector.tensor_mul(
                            ex[:, rel * CH:(rel + 1) * CH],
                            ex[:, rel * CH:(rel + 1) * CH],
                            mp_sb[:, uidx, :])
                    nc.tensor.matmul(
                        raw[:, o:o + w],
                        v_all[:, kt, h * (DK + 1):(h + 1) * (DK + 1)],
                        ex[:, 0:w],
                        start=first, stop=False,
                        skip_group_check=True)
                    first = False
                # normalize -> attnT_cat
                rec = recp.tile([1, QB], F32, tag="rec")
                scr = recp.tile([1, QB], F32, tag="scr")
                den = recp.tile([1, QB], F32, tag="den")
                if guard_empty_rows:
                    nc.vector.tensor_scalar_max(den[:], raw[DK:DK + 1, :], 1e-30)
                else:
                    nc.vector.tensor_copy(den[:], raw[DK:DK + 1, :])
                nc.vector.reciprocal_approx_accurate(
                    out=rec[:], in_=den[:], scratch=scr[:])
                recb = recp.tile([DK, QB], F32, tag="recb")
                nc.gpsimd.partition_broadcast(recb[:], rec[:])
                nc.vector.tensor_mul(
                    att[hp:hp + DK, hm, :], raw[0:DK, :], recb[:])

            # out-proj for this q-block
            for mo in range(D // CH):
                ps = ps_mm.tile([CH, QB], F32, tag="mm")
                for ct in range(MT):
                    nc.tensor.matmul(
                        ps[:], wo_all[:, ct, mo * CH:(mo + 1) * CH],
                        att[:, ct, :], start=(ct == 0), stop=(ct == MT - 1),
                        skip_group_check=True)
                ot = outp.tile([CH, QB], F32, tag="ot")
                nc.vector.tensor_copy(out=ot[:], in_=ps[:])
                nc.sync.dma_start(
                    outT.ap()[mo * CH:(mo + 1) * CH, qb * QB:(qb + 1) * QB],
                    ot[:])

    nc.compile()
    return nc


_CACHE = {}
LAST_RESULTS = None


def _install_ntff_shim():
    """Provide antenv.axon_hooks (NTFF profiling) when the image lacks it."""
    import sys, types, ctypes, contextlib
    if "antenv.axon_hooks" in sys.modules:
        return
    import antenv
    mod = types.ModuleType("antenv.axon_hooks")
    state = {"hook": None}
    mod.set_axon_ntff_profile_hook = lambda h: state.__setitem__("hook", h)
    mod.get_axon_ntff_profile_hook = lambda: state["hook"]
    sys.modules["antenv.axon_hooks"] = mod
    antenv.axon_hooks = mod
    try:
        lib = ctypes.CDLL("/opt/axon/libaxon_pjrt.so")
    except OSError:
        return
    if not hasattr(lib, "axon_start_nrt_profile"):
        return
    lib.axon_start_nrt_profile.argtypes = [
        ctypes.POINTER(ctypes.c_int64), ctypes.c_size_t]
    lib.axon_start_nrt_profile.restype = ctypes.c_int64
    lib.axon_stop_nrt_profile.argtypes = [ctypes.c_char_p]
    lib.axon_stop_nrt_profile.restype = ctypes.c_int64

    @contextlib.contextmanager
    def _hook(output_dir, device_ids):
        import jax
        jax.devices()
        if device_ids:
            ids = (ctypes.c_int64 * len(device_ids))(*device_ids)
            rc = lib.axon_start_nrt_profile(ids, len(device_ids))
        else:
            rc = lib.axon_start_nrt_profile(None, 0)
        if rc != 0:
            raise RuntimeError(f"axon_start_nrt_profile rc={rc}")
        try:
            yield
        finally:
            n = lib.axon_stop_nrt_profile(str(output_dir).encode())
            print(f"profile: {n} ntff file(s) in {output_dir}", file=sys.stderr)

    state["hook"] = _hook


def _get_nc(mask2d):
    key = hash(mask2d.tobytes())
    if key not in _CACHE:
        plans, patterns = _plan_from_mask(mask2d)
        # guard against fully-masked rows (reference maps softmax NaN -> 0)
        valid_any = (~mask2d).any(axis=1)
        guard = bool((~valid_any).any())
        _CACHE[key] = (_build(plans, len(patterns), guard), patterns)
    return _CACHE[key]


def kernel(query, key, value, mask, Wq, bq, Wk, bk, Wv, bv, Wo, bo):
    from concourse.bass_utils import run_bass_kernel_spmd

    query = np.asarray(query, dtype=np.float32)
    key_ = np.asarray(key, dtype=np.float32)
    value = np.asarray(value, dtype=np.float32)
    mask2d = np.asarray(mask, dtype=bool).reshape(S, S)
    Wq = np.asarray(Wq, dtype=np.float32)
    Wk = np.asarray(Wk, dtype=np.float32)
    Wv = np.asarray(Wv, dtype=np.float32)
    Wo = np.asarray(Wo, dtype=np.float32)
    bq = np.asarray(bq, dtype=np.float32)
    bk = np.asarray(bk, dtype=np.float32)
    bv = np.asarray(bv, dtype=np.float32)
    bo = np.asarray(bo, dtype=np.float32)

    nc, patterns = _get_nc(mask2d)

    n_pat = len(patterns)
    if n_pat:
        mp = np.empty((CH, n_pat * CH), np.float32)
        for u, pat in enumerate(patterns):
            mp[:, u * CH:(u + 1) * CH] = pat
        mp = mp.astype(ml_dtypes.bfloat16)
    ones_row = np.ones((1, QB), ml_dtypes.bfloat16)
    ones_cols = np.ones((CH, NKT * HL), ml_dtypes.bfloat16)

    in_maps = []
    for c in range(NCORES):
        b, g = divmod(c, 2)
        gsl = slice(DL * g, DL * (g + 1))
        m = {
            "xq_t": _bf16(query[b].T),
            "xk_t": _bf16(key_[b].T),
            "xv_t": _bf16(value[b].T),
            "wq_t": _bf16(Wq[gsl].T * 0.125),
            "wk_t": _bf16(Wk[gsl].T),
            "wv_t": _bf16(Wv[gsl].T),
            "wo_t": _bf16(Wo[:, gsl].T),
            "bq8": _bf16(bq[gsl].reshape(1, DL) * 0.125),
            "bk": _bf16(bk[gsl].reshape(1, DL)),
            "bv": _bf16(bv[gsl].reshape(1, DL)),
            "ones_row": ones_row,
            "ones_cols": ones_cols,
        }
        if n_pat:
            m["maskp"] = mp
        in_maps.append(m)

    import os
    kwargs = {}
    if os.environ.get("BASS_MHA_TRACE"):
        _install_ntff_shim()
        kwargs = dict(trace=True, trace_cores=[0])
    res = run_bass_kernel_spmd(nc, in_maps, core_ids=list(range(NCORES)), **kwargs)
    global LAST_RESULTS
    LAST_RESULTS = res

    out = np.empty((B, S, D), np.float32)
    for b in range(B):
        acc = res.results[2 * b]["outT"] + res.results[2 * b + 1]["outT"]
        out[b] = acc.T + bo[None, :]
    return out
